# revision 1
# baseline (speedup 1.0000x reference)
"""nn_MultiHeadAttention (B=2, S=2048, D=2048, H=16) on 8 NeuronCores.

The reference module splits heads with a plain reshape (no transpose):
    Q = (x @ Wq.T).reshape(B, H, S, Dh)
so head h attends over ROWS [128h, 128h+128) of Qmat = x @ Wq.T, with
attention position s = 16a + r mapping to (row 128h + a, feature slice
[128r, 128r+128)).  The merge DOES transpose (standard), so
    y = sum_h outh @ Wo[:, 128h:128h+128].T.

Sharding: core c handles batch b=c//4 and head-group g=c%4 (heads
4g..4g+3, i.e. Qmat/Kmat/Vmat rows [512g, 512g+512) of its batch).  Each
core computes those projection row-slices (against the FULL Wq/Wk/Wv,
streamed in bf16), causal attention in the scrambled index space, and a
partial output projection against its column slice of Wo.  The host sums
the 4 bf16 partials per batch in fp32.

Per-head layout (everything stays on-chip, no DRAM scratch):
  Phase A writes Q/K/V directly into per-head SBUF tiles [dh, a, r]
  (transposed projections; 4 strided PSUM-scatter copies per tile).
  Attention is blocked by 'a' (= s//16), where causality is monotone:
  k-tiles are a''-octets x all 16 r' (128 partitions), q-blocks are
  32-wide a-ranges x all r (512 columns).  Tiles fully below the diagonal
  need no mask, tiles above are skipped entirely (true causal flop
  count); 4 precomputed [128, 512] masks cover the diagonal.  The k-major
  V tiles for attn@V come from 16 cheap PE transposes per head.  Softmax
  needs no max-subtraction (|scores/128| < ~0.6); denominators come from
  an all-ones-lhsT matmul per k-octet (accumulated in PSUM, already
  partition-broadcast), then one DVE reciprocal + multiply per q-block.

Matmuls run in bf16 with fp32 PSUM accumulation (measured end-to-end
scale-relative error ~3e-3 vs the fp32 reference).
"""

import sys

try:
    import concourse.bass as bass
except ImportError:  # harness may not have the repo on PYTHONPATH
    for p in ("/root/.axon_site", "/root/.axon_site/_ro/trn_rl_repo",
              "/root/.axon_site/_ro/pypackages", "/opt/trn_rl_repo"):
        if p not in sys.path:
            sys.path.append(p)
    import concourse.bass as bass

import numpy as np

import concourse.mybir as mybir
import concourse.tile as tile
from concourse.bass_utils import run_bass_kernel_spmd

F32 = mybir.dt.float32
F32R = mybir.dt.float32r
BF16 = mybir.dt.bfloat16
DT = BF16  # on-chip matmul dtype
AF = mybir.ActivationFunctionType

B = 2
S = 2048
DM = 2048
H = 16
DH = 128
N_CORES = 8
HPC = 4                 # heads per core
DL = HPC * DH           # 512: per-core row/col slice width
P = 128
QB = 512                # q-block width = 4 r-stripes x 128 a
N_DM = DM // P          # 16 contraction tiles
NR = 16                 # r-stripes per head


def _split_multi_waits(nc):
    """This container's walrus rejects >1 sync-wait per instruction.
    Hoist extra waits onto same-engine NoOps inserted just before."""
    ctr = 0
    for f in nc.m.functions:
        for bb in f.blocks:
            insts = bb.instructions
            fixes = []
            for idx, inst in enumerate(insts):
                si = inst.sync_info
                ow = list(si.on_wait) if si and si.on_wait else []
                if len(ow) > 1:
                    fixes.append((idx, inst, ow, si))
            for idx, inst, ow, si in reversed(fixes):
                inst.sync_info = mybir.SyncInfo(on_wait=ow[-1:], on_update=si.on_update)
                for w in reversed(ow[:-1]):
                    ctr += 1
                    nop = mybir.InstNoOp(
                        name=f"I-waitsplit-{ctr}", engine=inst.engine, ins=[], outs=[]
                    )
                    nop.sync_info = mybir.SyncInfo(on_wait=[w], on_update=[])
                    nc.register_instruction(nop, overwrite=True)
                    insts.insert(idx, nop)
    return ctr


def _build_nc():
    nc = bass.Bass(target_bir_lowering=False)

    xs_d = nc.dram_tensor("xs", [DM, DL], DT, kind="ExternalInput")    # x[b,rows].T
    wqt_d = nc.dram_tensor("wqt", [DM, DM], DT, kind="ExternalInput")  # Wq.T (full)
    wkt_d = nc.dram_tensor("wkt", [DM, DM], DT, kind="ExternalInput")
    wvt_d = nc.dram_tensor("wvt", [DM, DM], DT, kind="ExternalInput")
    wot_d = nc.dram_tensor("wot", [DL, DM], DT, kind="ExternalInput")  # Wo[:,slice].T
    mask_d = nc.dram_tensor("maskc", [4, P, QB], DT, kind="ExternalInput")
    ones_d = nc.dram_tensor("ones", [P, P], DT, kind="ExternalInput")
    ident_d = nc.dram_tensor("ident", [P, P], DT, kind="ExternalInput")
    yt_d = nc.dram_tensor("yt", [DM, S], DT, kind="ExternalOutput")    # partial y[b].T

    yt_t3 = yt_d.rearrange("(o p) s -> p o s", p=P)

    with tile.TileContext(nc) as tc:
        with (
            tc.tile_pool(name="stage", bufs=6) as stage,
            tc.tile_pool(name="small", bufs=4) as small,
            tc.tile_pool(name="proj", bufs=HPC) as proj,
            tc.tile_pool(name="ps_s", bufs=3, space="PSUM") as ps_s,
            tc.tile_pool(name="ps_o", bufs=3, space="PSUM") as ps_o,
            tc.tile_pool(name="ps_l", bufs=2, space="PSUM") as ps_l,
            nc.allow_low_precision(reason="bf16 attention kernel"),
        ):
            # per-head projection tiles in [dh, a, r] layout, filled by phase A
            qt2 = [proj.tile([P, P, NR], DT, tag="qt2", name=f"qt2_{i}") for i in range(HPC)]
            kt2 = [proj.tile([P, P, NR], DT, tag="kt2", name=f"kt2_{i}") for i in range(HPC)]
            vt2 = [proj.tile([P, P, NR], DT, tag="vt2", name=f"vt2_{i}") for i in range(HPC)]

            # ---- phase A: projection row-slices straight into SBUF ----
            with (
                tc.tile_pool(name="xpool", bufs=1) as xpool,
                tc.tile_pool(name="wqk", bufs=8) as wqk,
            ):
                xs_t = xpool.tile([P, N_DM, DL], DT, tag="x")
                xs_t3 = xs_d.rearrange("(o p) s -> p o s", p=P)
                for i in range(4):
                    nc.sync.dma_start(
                        xs_t[:, 4 * i:4 * (i + 1), :], xs_t3[:, 4 * i:4 * (i + 1), :]
                    )
                for w_d, dst in ((wvt_d, vt2), (wkt_d, kt2), (wqt_d, qt2)):
                    w_t3 = w_d.rearrange("(o p) d -> p o d", p=P)
                    for rt in range(NR):
                        w_t = wqk.tile([P, N_DM, P], DT, tag="wqk")
                        nc.sync.dma_start(w_t[:], w_t3[:, :, rt * P:(rt + 1) * P])
                        psum = ps_s.tile([P, QB], F32, tag="ps")
                        for dm in range(N_DM):
                            nc.tensor.matmul(
                                psum[:], lhsT=w_t[:, dm, :], rhs=xs_t[:, dm, :],
                                start=(dm == 0), stop=(dm == N_DM - 1),
                            )
                        # scatter into per-head [dh, a, r] tiles (r-strided)
                        for hl in range(HPC):
                            nc.any.tensor_copy(
                                dst[hl][:, :, rt], psum[:, hl * P:(hl + 1) * P]
                            )

            # ---- phase B: attention per head (scrambled index space) ----
            with (
                tc.tile_pool(name="bconst", bufs=1) as bconst,
                tc.tile_pool(name="hpool", bufs=3) as hpool,
                tc.tile_pool(name="atpool", bufs=8) as atpool,
                tc.tile_pool(name="attt", bufs=HPC) as attt_pool,
            ):
                ones_t = bconst.tile([P, P], DT, tag="ones")
                nc.sync.dma_start(ones_t[:], ones_d[:])
                mask_t = bconst.tile([P, 4, QB], DT, tag="mask")
                nc.sync.dma_start(mask_t[:], mask_d.rearrange("c p q -> p c q"))
                ident_t = bconst.tile([P, P], DT, tag="ident")
                nc.sync.dma_start(ident_t[:], ident_d[:])

                att_tiles = []
                for hl in range(HPC):
                    # vk: k-major V tiles via PE transpose, partition=(a'',r')
                    vk_h = hpool.tile([P, NR, P], DT, tag="v")    # [(a'' r'), m, dh]

                    def emit_vk(m, hl=hl, vk_h=vk_h):
                        ps_t = ps_o.tile([P, P], DT, tag="po")
                        nc.tensor.transpose(
                            ps_t[:], vt2[hl][:, 8 * m:8 * (m + 1), :], ident_t[:]
                        )
                        nc.any.tensor_copy(vk_h[:, m, :], ps_t[:])

                    att_h = attt_pool.tile([P, P, NR], DT, tag="attT")  # [dh, a, r]
                    att_tiles.append(att_h)

                    for qb in range(4):
                        a0 = 32 * qb
                        nk = 4 * (qb + 1)   # k-octets 0..nk-1
                        for m in range(4 * qb, nk):
                            emit_vk(m)
                        psum_o = ps_o.tile([P, QB], F32, tag="po")
                        psum_l = ps_l.tile([P, QB], F32, tag="pl")
                        ats = [None] * nk

                        def emit_scores(m):
                            psum_s = ps_s.tile([P, QB], F32, tag="ps")
                            nc.tensor.matmul(
                                psum_s[:],
                                lhsT=kt2[hl][:, 8 * m:8 * (m + 1), :],
                                rhs=qt2[hl][:, a0:a0 + 32, :],
                                start=True, stop=True,
                            )
                            at = atpool.tile([P, QB], DT, tag="at")
                            nc.scalar.activation(at[:], psum_s[:], AF.Exp, scale=1.0 / DH)
                            if m >= 4 * qb:
                                nc.vector.tensor_mul(at[:], at[:], mask_t[:, m - 4 * qb, :])
                            ats[m] = at

                        def emit_ov(m):
                            nc.tensor.matmul(
                                psum_o[:],
                                lhsT=vk_h[:, m, :], rhs=ats[m][:],
                                start=(m == 0), stop=(m == nk - 1),
                            )
                            nc.tensor.matmul(
                                psum_l[:],
                                lhsT=ones_t[:, :], rhs=ats[m][:],
                                start=(m == 0), stop=(m == nk - 1),
                            )

                        DEPTH = 2
                        for m in range(nk):
                            emit_scores(m)
                            if m >= DEPTH:
                                emit_ov(m - DEPTH)
                        for m in range(nk - DEPTH, nk):
                            emit_ov(m)

                        # normalize: att = psum_o * (1/l)
                        rcb = small.tile([P, QB], F32, tag="rcb")
                        nc.vector.reciprocal(rcb[:], psum_l[:])
                        nc.vector.tensor_mul(
                            att_h[:, a0:a0 + 32, :],
                            psum_o[:].rearrange("p (a r) -> p a r", a=32),
                            rcb[:].rearrange("p (a r) -> p a r", a=32),
                        )

                # ---- phase C: partial output projection yT = WoT.T @ attT ----
                with tc.tile_pool(name="wop", bufs=1) as wop:
                    wot_t = wop.tile([P, HPC, DM], DT, tag="wo")
                    nc.sync.dma_start(
                        wot_t[:],
                        wot_d.rearrange("(hl p) d -> p hl d", p=P),
                    )
                    att_flat = [
                        t[:].rearrange("p a r -> p (a r)") for t in att_tiles
                    ]
                    for ot in range(N_DM):
                        for sb in range(4):
                            psum = ps_s.tile([P, QB], F32, tag="ps")
                            for hl in range(HPC):
                                nc.tensor.matmul(
                                    psum[:],
                                    lhsT=wot_t[:, hl, ot * P:(ot + 1) * P],
                                    rhs=att_flat[hl][:, sb * QB:(sb + 1) * QB],
                                    start=(hl == 0), stop=(hl == HPC - 1),
                                )
                            st = stage.tile([P, QB], DT, tag="ystage")
                            nc.any.tensor_copy(st[:], psum[:])
                            nc.sync.dma_start(yt_t3[:, ot, sb * QB:(sb + 1) * QB], st[:])

    _split_multi_waits(nc)
    return nc


_NC = None


def _make_masks():
    # a-blocked causal masks for diagonal tiles, (a-outer, r-inner) order:
    # k partition index p = a''*16 + r';  q column index j = a_rel*16 + r
    # allow k <= q:  16*(8*mi + a'') + r'  <=  16*a_rel + r
    k_lin = (16 * np.arange(8)[:, None] + np.arange(NR)[None, :]).reshape(-1)   # 128
    q_lin = (16 * np.arange(32)[:, None] + np.arange(NR)[None, :]).reshape(-1)  # 512
    out = np.empty((4, P, QB), dtype=np.float32)
    for mi in range(4):
        out[mi] = ((k_lin[:, None] + 128 * mi) <= q_lin[None, :]).astype(np.float32)
    return out


def kernel(x, Wq, Wk, Wv, Wo, _want_trace=False, **_trace_kw):
    global _NC
    if _NC is None:
        _NC = _build_nc()
    nc = _NC

    import ml_dtypes
    bf16 = ml_dtypes.bfloat16

    x = np.asarray(x, dtype=np.float32)
    wqt = np.ascontiguousarray(np.asarray(Wq, dtype=np.float32).T).astype(bf16)
    wkt = np.ascontiguousarray(np.asarray(Wk, dtype=np.float32).T).astype(bf16)
    wvt = np.ascontiguousarray(np.asarray(Wv, dtype=np.float32).T).astype(bf16)
    Wo = np.asarray(Wo, dtype=np.float32)
    masks = _make_masks().astype(bf16)
    ones = np.ones((P, P), dtype=bf16)
    ident = np.eye(P, dtype=np.float32).astype(bf16)

    in_maps = []
    for c in range(N_CORES):
        b, g = divmod(c, HPC)
        sl = slice(g * DL, (g + 1) * DL)
        in_maps.append({
            "xs": np.ascontiguousarray(x[b, sl, :].T).astype(bf16),
            "wqt": wqt,
            "wkt": wkt,
            "wvt": wvt,
            "wot": np.ascontiguousarray(Wo[:, sl].T).astype(bf16),
            "maskc": masks,
            "ones": ones,
            "ident": ident,
        })

    res = run_bass_kernel_spmd(
        nc, in_maps, list(range(N_CORES)),
        trace=_want_trace, **_trace_kw,
    )

    y = np.empty((B, S, DM), dtype=np.float32)
    for b in range(B):
        acc = res.results[HPC * b]["yt"].astype(np.float32)
        for g in range(1, HPC):
            acc += res.results[HPC * b + g]["yt"].astype(np.float32)
        y[b] = acc.T
    if _want_trace:
        return y, res
    return y



# revision 14
# speedup vs baseline: 1.1053x; 1.1053x over previous
"""nn_MultiHeadAttention (B=2, S=2048, D=2048, H=16) on 8 NeuronCores.

The reference module splits heads with a plain reshape (no transpose):
    Q = (x @ Wq.T).reshape(B, H, S, Dh)
so head h attends over ROWS [128h, 128h+128) of Qmat = x @ Wq.T, with
attention position s' = 16a + r mapping to (row 128h + a, feature slice
[128r, 128r+128)).  The merge DOES transpose (standard), so
    y = sum_h outh @ Wo[:, 128h:128h+128].T.

Sharding: core c handles batch b=c//4 and head-group g=c%4 (heads
4g..4g+3, i.e. tokens [512g, 512g+512) of its batch).  Each core
computes those projection row-slices against the FULL Wq/Wk/Wv, causal
attention in the scrambled index space, and a partial output projection
against its column slice of Wo.  The host sums the 4 partials per batch
in fp32 and unscrambles the column order.

Precision strategy (validated numerically, rel-err ~2.5e-3 vs fp32):
  * Q/K projections run in fp8e4m3 with DoubleRow perf mode (two
    128-partition contraction subtiles per matmul = 2x PE throughput).
    Scales: x*32, W*1024 (both < 240 max-normal), descaled 2^-15 at the
    PSUM->SBUF copy.  Softmax forgives the ~2.6% Q/K quantization.
  * Everything else runs fp16 (NOT bf16): same PE speed, 4x lower
    rounding error, and 2x/4x DVE throughput for the elementwise work.
  * V path / attention weights / output projection must NOT be fp8
    (measured 2.4e-2..3.8e-2 rel-err = over the 2e-2 gate).

Layout: projections stored as single tiles [dh=128, r=16, 512 tokens]
filled by ONE copy per 512-wide PSUM stripe (no per-head scatter).
Head hl's tiles are column slices [:, :, 128*hl : 128*hl+128].  Scores
use k-octets (free index i = 8r' + a'') against 512-wide q-blocks
(j = 32r + a_rel); causal masks precomputed on host for this order.
Softmax denominators: DVE accumulates the exp'd octets per q-block
(fp16, 4x mode), then a single ones-matmul per q-block broadcasts the
partition sum - removing ~30us of ones-matmuls from the PE stream.
Per-q-block finalize (last attn@V pair, ones-matmul, reciprocal,
normalize) is deferred until the next q-block's first score pair so the
PE never stalls on the DVE chain.  exp runs on 2-octet batches to halve
the activation-engine instruction overhead (ACT is the phase-B
co-bottleneck).  The output projection reuses phase-B PSUM tiles and
streams each [128,512] block to DRAM as it completes.
"""

import sys

try:
    import concourse.bass as bass
except ImportError:  # harness may not have the repo on PYTHONPATH
    for p in ("/root/.axon_site", "/root/.axon_site/_ro/trn_rl_repo",
              "/root/.axon_site/_ro/pypackages", "/opt/trn_rl_repo"):
        if p not in sys.path:
            sys.path.append(p)
    import concourse.bass as bass

import numpy as np

import concourse.mybir as mybir
import concourse.tile as tile
from concourse.bass_utils import run_bass_kernel_spmd

F32 = mybir.dt.float32
F16 = mybir.dt.float16
F8 = mybir.dt.float8e4
AF = mybir.ActivationFunctionType
DR = mybir.MatmulPerfMode.DoubleRow

B = 2
S = 2048
DM = 2048
H = 16
DH = 128
N_CORES = 8
HPC = 4                 # heads per core
DL = HPC * DH           # 512: per-core token-slice width
P = 128
QB = 512                # q-block width = 32 a x 16 r
N_DM = DM // P          # 16 contraction subtiles
NR = 16                 # r-stripes per head

SX = 32.0               # fp8 scale on x       (|x|max ~5.3  -> ~170 < 240)
SW = 1024.0             # fp8 scale on Wq/Wk   (|W|max ~0.12 -> ~120 < 240)
DESCALE = 1.0 / (SX * SW)


def _split_multi_waits(nc):
    """This container's walrus rejects >1 sync-wait per instruction.
    Hoist extra waits onto same-engine NoOps inserted just before."""
    ctr = 0
    for f in nc.m.functions:
        for bb in f.blocks:
            insts = bb.instructions
            fixes = []
            for idx, inst in enumerate(insts):
                si = inst.sync_info
                ow = list(si.on_wait) if si and si.on_wait else []
                if len(ow) > 1:
                    fixes.append((idx, inst, ow, si))
            for idx, inst, ow, si in reversed(fixes):
                inst.sync_info = mybir.SyncInfo(on_wait=ow[-1:], on_update=si.on_update)
                for w in reversed(ow[:-1]):
                    ctr += 1
                    nop = mybir.InstNoOp(
                        name=f"I-waitsplit-{ctr}", engine=inst.engine, ins=[], outs=[]
                    )
                    nop.sync_info = mybir.SyncInfo(on_wait=[w], on_update=[])
                    nc.register_instruction(nop, overwrite=True)
                    insts.insert(idx, nop)
    return ctr


def _build_nc():
    nc = bass.Bass(target_bir_lowering=False)

    x8_d = nc.dram_tensor("x8", [DM, DL], F8, kind="ExternalInput")      # x[b,sl].T * 32
    x16_d = nc.dram_tensor("x16", [DM, DL], F16, kind="ExternalInput")   # x[b,sl].T
    wq8_d = nc.dram_tensor("wq8", [DM, DM], F8, kind="ExternalInput")    # Wq.T * 1024
    wk8_d = nc.dram_tensor("wk8", [DM, DM], F8, kind="ExternalInput")
    wv_d = nc.dram_tensor("wv16", [DM, DM], F16, kind="ExternalInput")   # Wv.T
    wot_d = nc.dram_tensor("wot16", [DL, DM], F16, kind="ExternalInput")  # Wo[:,sl].T
    mask_d = nc.dram_tensor("maskc", [4, P, QB], F16, kind="ExternalInput")
    ones_d = nc.dram_tensor("ones", [P, P], F16, kind="ExternalInput")
    ident_d = nc.dram_tensor("ident", [P, P], F16, kind="ExternalInput")
    yt_d = nc.dram_tensor("yt", [DM, S], F16, kind="ExternalOutput")     # partial y[b].T

    yt_t3 = yt_d.rearrange("(o p) s -> p o s", p=P)

    with tile.TileContext(nc) as tc:
        with (
            tc.tile_pool(name="proj", bufs=1) as proj,
            tc.tile_pool(name="bconst", bufs=1) as bconst,
            tc.tile_pool(name="attp", bufs=HPC) as attp,
            tc.tile_pool(name="wop", bufs=1) as wop,
            tc.tile_pool(name="stg", bufs=4) as stg,
            nc.allow_low_precision(reason="fp8/fp16 attention kernel"),
        ):
            # projection tiles [dh, r, token]; head hl = cols [128hl, 128hl+128)
            qt = proj.tile([P, DL, NR], F16, tag="qt")
            kt = proj.tile([P, DL, NR], F16, tag="kt")
            vt = proj.tile([P, DL, NR], F16, tag="vt")
            # normalized attention outputs per head [dh, qb, j]
            att = [attp.tile([P, 4, QB], F16, tag="att", name=f"att{i}")
                   for i in range(HPC)]
            ones_t = bconst.tile([P, P], F16, tag="ones")
            ident_t = bconst.tile([P, P], F16, tag="ident")
            mask_t = bconst.tile([P, 4, QB], F16, tag="mask")
            wot_t = wop.tile([P, HPC, DM], F16, tag="wo")

            # ---- phase A: projections straight into SBUF ----
            with (
                tc.tile_pool(name="xp", bufs=1) as xp,
                tc.tile_pool(name="wp", bufs=6) as wp,
                tc.tile_pool(name="ps_a", bufs=4, space="PSUM") as ps_a,
            ):
                x8_t = xp.tile([P, N_DM, DL], F8, tag="x8")
                x16_t = xp.tile([P, N_DM, DL], F16, tag="x16")
                nc.sync.dma_start(x8_t[:], x8_d.rearrange("(o p) s -> p o s", p=P))

                for w_i, (w_d, w_dt, dst) in enumerate((
                    (wq8_d, F8, qt),
                    (wk8_d, F8, kt),
                    (wv_d, F16, vt),
                )):
                    w_t3 = w_d.rearrange("(o p) d -> p o d", p=P)
                    if w_i == 1:
                        # x16 queued behind the Q-phase weight stream
                        nc.sync.dma_start(
                            x16_t[:], x16_d.rearrange("(o p) s -> p o s", p=P)
                        )
                    for rt in range(NR):
                        w_t = wp.tile([P, N_DM, P], w_dt, tag=f"w{w_i}")
                        nc.sync.dma_start(w_t[:], w_t3[:, :, rt * P:(rt + 1) * P])
                        psum = ps_a.tile([P, QB], F32, tag="pa")
                        if w_dt == F8:
                            for d in range(8):
                                nc.tensor.matmul(
                                    psum[:],
                                    lhsT=w_t[:, 2 * d:2 * d + 2, :],
                                    rhs=x8_t[:, 2 * d:2 * d + 2, :],
                                    start=(d == 0), stop=(d == 7),
                                    perf_mode=DR,
                                )
                        else:
                            for d in range(N_DM):
                                nc.tensor.matmul(
                                    psum[:],
                                    lhsT=w_t[:, d, :], rhs=x16_t[:, d, :],
                                    start=(d == 0), stop=(d == N_DM - 1),
                                )
                        # one copy per stripe, spread across engines
                        if w_i == 0:
                            nc.vector.tensor_scalar_mul(dst[:, :, rt], psum[:], DESCALE)
                        elif w_i == 1:
                            nc.scalar.mul(dst[:, :, rt], psum[:], DESCALE)
                        else:
                            nc.scalar.copy(dst[:, :, rt], psum[:])

            # constants + Wo stream in behind the weight DMAs
            nc.sync.dma_start(ident_t[:], ident_d[:])
            nc.sync.dma_start(ones_t[:], ones_d[:])
            nc.sync.dma_start(mask_t[:], mask_d.rearrange("c p q -> p c q"))
            nc.sync.dma_start(wot_t[:], wot_d.rearrange("(hl p) d -> p hl d", p=P))

            # ---- phase B + C ----
            with (
                tc.tile_pool(name="atp", bufs=2) as atp,
                tc.tile_pool(name="accp", bufs=2) as accp,
                tc.tile_pool(name="vkp", bufs=2) as vkp,
                tc.tile_pool(name="rcp", bufs=2) as rcp,
                tc.tile_pool(name="ps2", bufs=2, space="PSUM") as ps2p,
                tc.tile_pool(name="ps_o", bufs=2, space="PSUM") as ps_op,
                tc.tile_pool(name="ps_l", bufs=1, space="PSUM") as ps_lp,
                tc.tile_pool(name="ps_t", bufs=1, space="PSUM") as ps_tp,
            ):
                pending = []

                def flush_pending():
                    while pending:
                        pending.pop(0)()

                def emit_tr4(vk_dst, src_hl, m0):
                    # k-major V tiles via PE transpose: vk[i=8r'+a'', m, dh].
                    # 4 transposes share one PSUM bank; one DVE copy drains it
                    # (GPSIMD cannot read PSUM on this target).
                    ps_t = ps_tp.tile([P, 4, P], F16, tag="pt")
                    for k in range(4):
                        cc = src_hl * P + 8 * (m0 + k)
                        nc.tensor.transpose(
                            ps_t[:, k, :], vt[:, cc:cc + 8, :], ident_t[:]
                        )
                    nc.vector.tensor_copy(vk_dst[:, m0:m0 + 4, :], ps_t[:])

                vk = vkp.tile([P, NR, P], F16, tag="vk", name="vk0")
                emit_tr4(vk, 0, 0)

                for hl in range(HPC):
                    c0 = hl * P
                    vk_next = (vkp.tile([P, NR, P], F16, tag="vk",
                                        name=f"vk{hl + 1}")
                               if hl + 1 < HPC else None)

                    for qb in range(4):
                        nk = 4 * qb + 4
                        a0 = 32 * qb
                        at = atp.tile([P, NR, QB], F16, tag="at")
                        acc = accp.tile([P, QB], F16, tag="acc")
                        psum_o = ps_op.tile([P, QB], F32, tag="po")

                        def emit_av(u, vk=vk, at=at, psum_o=psum_o, nk=nk):
                            nc.tensor.matmul(
                                psum_o[:], lhsT=vk[:, u, :], rhs=at[:, u, :],
                                start=(u == 0), stop=(u == nk - 1),
                            )

                        for t in range(nk // 2):
                            ps2 = ps2p.tile([P, 2, QB], F32, tag="ps2")
                            for u in (2 * t, 2 * t + 1):
                                nc.tensor.matmul(
                                    ps2[:, u - 2 * t, :],
                                    lhsT=kt[:, c0 + 8 * u:c0 + 8 * u + 8, :],
                                    rhs=qt[:, c0 + a0:c0 + a0 + 32, :],
                                    start=True, stop=True,
                                )
                            # future vk transposes ride the score stream:
                            # qb0 preps octets 4..7, qb1 octets 8..11,
                            # qb2 octets 12..15, qb3 the next head's 0..3
                            if t == 1:
                                if qb < 3:
                                    emit_tr4(vk, hl, 4 * (qb + 1))
                                elif vk_next is not None:
                                    emit_tr4(vk_next, hl + 1, 0)
                            if t == 1:
                                # previous q-block's finalize rides here so the
                                # PE has fresh independent work queued first
                                flush_pending()
                            if t >= 2:
                                emit_av(2 * t - 4)
                                emit_av(2 * t - 3)
                            nc.scalar.activation(
                                at[:, 2 * t:2 * t + 2, :], ps2[:], AF.Exp,
                                scale=1.0 / DH,
                            )
                            for u in (2 * t, 2 * t + 1):
                                if u >= 4 * qb:  # diagonal octet: causal mask
                                    nc.vector.tensor_mul(
                                        at[:, u, :], at[:, u, :],
                                        mask_t[:, u - 4 * qb, :],
                                    )
                            if t == 0:
                                nc.vector.tensor_add(
                                    acc[:], at[:, 0, :], at[:, 1, :]
                                )
                            else:
                                nc.vector.tensor_add(acc[:], acc[:], at[:, 2 * t, :])
                                nc.vector.tensor_add(
                                    acc[:], acc[:], at[:, 2 * t + 1, :]
                                )
                        emit_av(nk - 4)
                        emit_av(nk - 3)

                        def finalize(hl=hl, qb=qb, nk=nk, acc=acc,
                                     psum_o=psum_o, emit_av=emit_av):
                            emit_av(nk - 2)
                            emit_av(nk - 1)
                            psum_l = ps_lp.tile([P, QB], F32, tag="pl")
                            nc.tensor.matmul(
                                psum_l[:], lhsT=ones_t[:], rhs=acc[:],
                                start=True, stop=True,
                            )
                            rcb = rcp.tile([P, QB], F32, tag="rcb")
                            nc.vector.reciprocal(rcb[:], psum_l[:])
                            nc.vector.tensor_mul(
                                att[hl][:, qb, :], psum_o[:], rcb[:]
                            )

                        pending.append(finalize)

                    vk = vk_next

                # ---- phase C: partial yT = WoT.T @ att, reusing B psum ----
                for sb in range(4):
                    for ot in range(N_DM):
                        pool = ps_op if (ot + sb) % 2 == 0 else ps_lp
                        psc = pool.tile([P, QB], F32,
                                        tag="po" if pool is ps_op else "pl")
                        for hl2 in range(HPC):
                            nc.tensor.matmul(
                                psc[:],
                                lhsT=wot_t[:, hl2, ot * P:(ot + 1) * P],
                                rhs=att[hl2][:, sb, :],
                                start=(hl2 == 0), stop=(hl2 == HPC - 1),
                            )
                        if sb == 0 and ot == 0:
                            flush_pending()
                        st = stg.tile([P, QB], F16, tag="st")
                        if (ot + sb) % 2 == 0:
                            nc.vector.tensor_copy(st[:], psc[:])
                        else:
                            nc.scalar.copy(st[:], psc[:])
                        nc.sync.dma_start(
                            yt_t3[:, ot, sb * QB:(sb + 1) * QB], st[:]
                        )

    _split_multi_waits(nc)
    return nc


_NC = None


def _make_masks():
    # causal masks for diagonal octets in (a-outer, r-inner) index order:
    # k partition i = 16a'' + r';  q column j = 16a_rel + r  (== position
    # within the q-block, so yt columns come out in plain s' order)
    # allow: 16*(8*delta + a'') + r' <= 16*a_rel + r
    k_lin = (16 * np.arange(8)[:, None] + np.arange(NR)[None, :]).reshape(-1)
    q_lin = (16 * np.arange(32)[:, None] + np.arange(NR)[None, :]).reshape(-1)
    out = np.empty((4, P, QB), dtype=np.float32)
    for d in range(4):
        out[d] = ((k_lin[:, None] + P * d) <= q_lin[None, :]).astype(np.float32)
    return out


def kernel(x, Wq, Wk, Wv, Wo, _want_trace=False, **_trace_kw):
    global _NC
    if _NC is None:
        _NC = _build_nc()
    nc = _NC

    import ml_dtypes
    f8 = ml_dtypes.float8_e4m3
    f16 = np.float16

    x = np.asarray(x, dtype=np.float32)
    Wq = np.asarray(Wq, dtype=np.float32)
    Wk = np.asarray(Wk, dtype=np.float32)
    Wv = np.asarray(Wv, dtype=np.float32)
    Wo = np.asarray(Wo, dtype=np.float32)

    wq8 = np.ascontiguousarray(Wq.T * SW).astype(f8)
    wk8 = np.ascontiguousarray(Wk.T * SW).astype(f8)
    wv16 = np.ascontiguousarray(Wv.T).astype(f16)
    masks = _make_masks().astype(f16)
    ones = np.ones((P, P), dtype=f16)
    ident = np.eye(P, dtype=np.float32).astype(f16)

    in_maps = []
    for c in range(N_CORES):
        b, g = divmod(c, HPC)
        sl = slice(g * DL, (g + 1) * DL)
        xs = np.ascontiguousarray(x[b, sl, :].T)
        in_maps.append({
            "x8": np.ascontiguousarray(xs * SX).astype(f8),
            "x16": xs.astype(f16),
            "wq8": wq8,
            "wk8": wk8,
            "wv16": wv16,
            "wot16": np.ascontiguousarray(Wo[:, sl].T).astype(f16),
            "maskc": masks,
            "ones": ones,
            "ident": ident,
        })

    res = run_bass_kernel_spmd(
        nc, in_maps, list(range(N_CORES)),
        trace=_want_trace, **_trace_kw,
    )

    y = np.empty((B, S, DM), dtype=np.float32)
    for b in range(B):
        acc = res.results[HPC * b]["yt"].astype(np.float32)
        for g in range(1, HPC):
            acc += res.results[HPC * b + g]["yt"].astype(np.float32)
        y[b] = acc.T
    if _want_trace:
        return y, res
    return y


# revision 21
# speedup vs baseline: 1.1371x; 1.0288x over previous
"""nn_MultiHeadAttention (B=2, S=2048, D=2048, H=16) on 8 NeuronCores.

The reference module splits heads with a plain reshape (no transpose):
    Q = (x @ Wq.T).reshape(B, H, S, Dh)
so head h attends over ROWS [128h, 128h+128) of Qmat = x @ Wq.T, with
attention position s' = 16a + r mapping to (row 128h + a, feature slice
[128r, 128r+128)).  The merge DOES transpose (standard), so
    y = sum_h outh @ Wo[:, 128h:128h+128].T.

Sharding: core c handles batch b=c//4 and head-group g=c%4 (heads
4g..4g+3, i.e. tokens [512g, 512g+512) of its batch).  Each core
computes those projection row-slices against the FULL Wq/Wk/Wv, causal
attention in the scrambled index space, and a partial output projection
against its column slice of Wo.  The host sums the 4 partials per batch
in fp32 and unscrambles the column order.

Precision strategy (validated numerically, rel-err ~2.5e-3 vs fp32):
  * Q/K projections run in fp8e4m3 with DoubleRow perf mode (two
    128-partition contraction subtiles per matmul = 2x PE throughput).
    Scales: x*32, W*1024 (both < 240 max-normal), descaled 2^-15 at the
    PSUM->SBUF copy.  Softmax forgives the ~2.6% Q/K quantization.
  * Everything else runs fp16 (NOT bf16): same PE speed, 4x lower
    rounding error, and 2x/4x DVE throughput for the elementwise work.
  * V path / attention weights / output projection must NOT be fp8
    (measured 2.4e-2..3.8e-2 rel-err = over the 2e-2 gate).

Layout: projections stored as single tiles [dh=128, r=16, 512 tokens]
filled by ONE copy per 512-wide PSUM stripe (no per-head scatter).
Head hl's tiles are column slices [:, :, 128*hl : 128*hl+128].  Scores
use k-octets (free index i = 8r' + a'') against 512-wide q-blocks
(j = 32r + a_rel); causal masks precomputed on host for this order.
Softmax denominators: DVE accumulates the exp'd octets per q-block
(fp16, 4x mode), then a single ones-matmul per q-block broadcasts the
partition sum - removing ~30us of ones-matmuls from the PE stream.
Per-q-block finalize (last attn@V pair, ones-matmul, reciprocal,
normalize) is deferred until the next q-block's first score pair so the
PE never stalls on the DVE chain.  exp runs on 2-octet batches to halve
the activation-engine instruction overhead (ACT is the phase-B
co-bottleneck).  The output projection reuses phase-B PSUM tiles and
streams each [128,512] block to DRAM as it completes.
"""

import sys

try:
    import concourse.bass as bass
except ImportError:  # harness may not have the repo on PYTHONPATH
    for p in ("/root/.axon_site", "/root/.axon_site/_ro/trn_rl_repo",
              "/root/.axon_site/_ro/pypackages", "/opt/trn_rl_repo"):
        if p not in sys.path:
            sys.path.append(p)
    import concourse.bass as bass

import numpy as np

import concourse.mybir as mybir
import concourse.tile as tile
from concourse.bass_utils import run_bass_kernel_spmd

F32 = mybir.dt.float32
F16 = mybir.dt.float16
F8 = mybir.dt.float8e4
AF = mybir.ActivationFunctionType
DR = mybir.MatmulPerfMode.DoubleRow

B = 2
S = 2048
DM = 2048
H = 16
DH = 128
N_CORES = 8
HPC = 4                 # heads per core
DL = HPC * DH           # 512: per-core token-slice width
P = 128
QB = 512                # q-block width = 32 a x 16 r
N_DM = DM // P          # 16 contraction subtiles
NR = 16                 # r-stripes per head

SX = 32.0               # fp8 scale on x       (|x|max ~5.3  -> ~170 < 240)
SW = 1024.0             # fp8 scale on Wq/Wk   (|W|max ~0.12 -> ~120 < 240)
DESCALE = 1.0 / (SX * SW)


def _split_multi_waits(nc):
    """This container's walrus rejects >1 sync-wait per instruction.
    Hoist extra waits onto same-engine NoOps inserted just before."""
    ctr = 0
    for f in nc.m.functions:
        for bb in f.blocks:
            insts = bb.instructions
            fixes = []
            for idx, inst in enumerate(insts):
                si = inst.sync_info
                ow = list(si.on_wait) if si and si.on_wait else []
                if len(ow) > 1:
                    fixes.append((idx, inst, ow, si))
            for idx, inst, ow, si in reversed(fixes):
                inst.sync_info = mybir.SyncInfo(on_wait=ow[-1:], on_update=si.on_update)
                for w in reversed(ow[:-1]):
                    ctr += 1
                    nop = mybir.InstNoOp(
                        name=f"I-waitsplit-{ctr}", engine=inst.engine, ins=[], outs=[]
                    )
                    nop.sync_info = mybir.SyncInfo(on_wait=[w], on_update=[])
                    nc.register_instruction(nop, overwrite=True)
                    insts.insert(idx, nop)
    return ctr


def _build_nc():
    nc = bass.Bass(target_bir_lowering=False)

    x8_d = nc.dram_tensor("x8", [DM, DL], F8, kind="ExternalInput")      # x[b,sl].T * 32
    x16_d = nc.dram_tensor("x16", [DM, DL], F16, kind="ExternalInput")   # x[b,sl].T
    wq8_d = nc.dram_tensor("wq8", [DM, DM], F8, kind="ExternalInput")    # Wq.T * 1024
    wk8_d = nc.dram_tensor("wk8", [DM, DM], F8, kind="ExternalInput")
    wv_d = nc.dram_tensor("wv16", [DM, DM], F16, kind="ExternalInput")   # Wv.T
    wot_d = nc.dram_tensor("wot16", [DL, DM], F16, kind="ExternalInput")  # Wo[:,sl].T
    mask_d = nc.dram_tensor("maskc", [4, P, QB], F16, kind="ExternalInput")
    ones_d = nc.dram_tensor("ones", [P, P], F16, kind="ExternalInput")
    ident_d = nc.dram_tensor("ident", [P, P], F16, kind="ExternalInput")
    yt_d = nc.dram_tensor("yt", [DM, S], F16, kind="ExternalOutput")     # partial y[b].T

    yt_t3 = yt_d.rearrange("(o p) s -> p o s", p=P)

    with tile.TileContext(nc) as tc:
        with (
            tc.tile_pool(name="proj", bufs=1) as proj,
            tc.tile_pool(name="bconst", bufs=1) as bconst,
            tc.tile_pool(name="attp", bufs=HPC) as attp,
            tc.tile_pool(name="wop", bufs=1) as wop,
            tc.tile_pool(name="stg", bufs=4) as stg,
            nc.allow_low_precision(reason="fp8/fp16 attention kernel"),
        ):
            # projection tiles [dh, r, token]; head hl = cols [128hl, 128hl+128)
            qt = proj.tile([P, DL, NR], F16, tag="qt")
            kt = proj.tile([P, DL, NR], F16, tag="kt")
            vt = proj.tile([P, DL, NR], F16, tag="vt")
            # normalized attention outputs per head [dh, qb, j]
            att = [attp.tile([P, 4, QB], F16, tag="att", name=f"att{i}")
                   for i in range(HPC)]
            ones_t = bconst.tile([P, P], F16, tag="ones")
            ident_t = bconst.tile([P, P], F16, tag="ident")
            mask_t = bconst.tile([P, 4, QB], F16, tag="mask")
            wot_t = wop.tile([P, HPC, DM], F16, tag="wo")

            # ---- phase A: projections straight into SBUF ----
            with (
                tc.tile_pool(name="xp", bufs=1) as xp,
                tc.tile_pool(name="wp", bufs=6) as wp,
                tc.tile_pool(name="ps_a", bufs=3, space="PSUM") as ps_a,
            ):
                x8_t = xp.tile([P, N_DM, DL], F8, tag="x8")
                x16_t = xp.tile([P, N_DM, DL], F16, tag="x16")
                x8_t3 = x8_d.rearrange("(o p) s -> p o s", p=P)
                nc.sync.dma_start(x8_t[:, 0:8, :], x8_t3[:, 0:8, :])
                nc.sync.dma_start(x8_t[:, 8:16, :], x8_t3[:, 8:16, :])
                x16_t3 = x16_d.rearrange("(o p) s -> p o s", p=P)

                for w_i, (w_d, w_dt, dst) in enumerate((
                    (wq8_d, F8, qt),
                    (wk8_d, F8, kt),
                    (wv_d, F16, vt),
                )):
                    w_t3 = w_d.rearrange("(o p) d -> p o d", p=P)
                    # stripe PAIRS share a 2-bank psum tile so the strided
                    # scatter copy writes 4-byte token units (the 2-byte
                    # strided write pattern measured 3x slower on hw)
                    for rp in range(NR // 2):
                        if w_i == 1 and rp < 4:
                            # x16 streamed in chunks between K stripes so it
                            # doesn't stall the K weight stream
                            nc.sync.dma_start(
                                x16_t[:, 4 * rp:4 * rp + 4, :],
                                x16_t3[:, 4 * rp:4 * rp + 4, :],
                            )
                        psum = ps_a.tile([P, 2, QB], F32, tag="pa")
                        for half in range(2):
                            rt = 2 * rp + half
                            w_t = wp.tile([P, N_DM, P], w_dt, tag=f"w{w_i}")
                            nc.sync.dma_start(
                                w_t[:], w_t3[:, :, rt * P:(rt + 1) * P]
                            )
                            if w_dt == F8:
                                for d in range(8):
                                    nc.tensor.matmul(
                                        psum[:, half, :],
                                        lhsT=w_t[:, 2 * d:2 * d + 2, :],
                                        rhs=x8_t[:, 2 * d:2 * d + 2, :],
                                        start=(d == 0), stop=(d == 7),
                                        perf_mode=DR,
                                    )
                            else:
                                for d in range(N_DM):
                                    nc.tensor.matmul(
                                        psum[:, half, :],
                                        lhsT=w_t[:, d, :], rhs=x16_t[:, d, :],
                                        start=(d == 0), stop=(d == N_DM - 1),
                                    )
                        # one paired scatter copy, spread across engines
                        dst_ap = dst[:, :, 2 * rp:2 * rp + 2]
                        src_ap = psum[:].rearrange("p t c -> p c t")
                        if w_i == 0 or (w_i == 2 and rp % 2 == 0):
                            nc.vector.tensor_scalar_mul(
                                dst_ap, src_ap, DESCALE if w_i == 0 else 1.0
                            )
                        elif w_i == 1:
                            nc.scalar.mul(dst_ap, src_ap, DESCALE)
                        else:
                            nc.scalar.copy(dst_ap, src_ap)

            # constants + Wo stream in behind the weight DMAs
            nc.sync.dma_start(ident_t[:], ident_d[:])
            nc.sync.dma_start(ones_t[:], ones_d[:])
            nc.sync.dma_start(mask_t[:], mask_d.rearrange("c p q -> p c q"))
            nc.sync.dma_start(wot_t[:], wot_d.rearrange("(hl p) d -> p hl d", p=P))

            # ---- phase B + C ----
            with (
                tc.tile_pool(name="atp", bufs=2) as atp,
                tc.tile_pool(name="accp", bufs=2) as accp,
                tc.tile_pool(name="vkp", bufs=2) as vkp,
                tc.tile_pool(name="rcp", bufs=2) as rcp,
                tc.tile_pool(name="ps2", bufs=2, space="PSUM") as ps2p,
                tc.tile_pool(name="ps_o", bufs=2, space="PSUM") as ps_op,
                tc.tile_pool(name="ps_l", bufs=1, space="PSUM") as ps_lp,
                tc.tile_pool(name="ps_t", bufs=1, space="PSUM") as ps_tp,
            ):
                pending = []
                gqb = 0   # global q-block counter (for at-pool warmup)

                def flush_pending():
                    while pending:
                        pending.pop(0)()

                def emit_tr4(vk_dst, src_hl, m0):
                    # k-major V tiles via PE transpose: vk[i=8r'+a'', m, dh].
                    # 4 transposes share one PSUM bank; one DVE copy drains it
                    # (GPSIMD cannot read PSUM on this target).
                    ps_t = ps_tp.tile([P, 4, P], F16, tag="pt")
                    for k in range(4):
                        cc = src_hl * P + 8 * (m0 + k)
                        nc.tensor.transpose(
                            ps_t[:, k, :], vt[:, cc:cc + 8, :], ident_t[:]
                        )
                    nc.vector.tensor_copy(vk_dst[:, m0:m0 + 4, :], ps_t[:])

                vk = vkp.tile([P, NR, P], F16, tag="vk", name="vk0")
                emit_tr4(vk, 0, 0)

                for hl in range(HPC):
                    c0 = hl * P
                    vk_next = (vkp.tile([P, NR, P], F16, tag="vk",
                                        name=f"vk{hl + 1}")
                               if hl + 1 < HPC else None)

                    for qb in range(4):
                        nk = 4 * qb + 4
                        a0 = 32 * qb
                        at = atp.tile([P, NR, QB], F16, tag="at")
                        acc = accp.tile([P, QB], F16, tag="acc")
                        psum_o = ps_op.tile([P, QB], F32, tag="po")

                        def emit_av(u, vk=vk, at=at, psum_o=psum_o, nk=nk):
                            nc.tensor.matmul(
                                psum_o[:], lhsT=vk[:, u, :], rhs=at[:, u, :],
                                start=(u == 0), stop=(u == nk - 1),
                            )

                        for t in range(nk // 2):
                            # the last pair holds diagonal octets 4qb+2/4qb+3
                            # whose valid q-columns are [256, 512): compute
                            # only those, zero the dead half explicitly
                            rstr = t == nk // 2 - 1
                            lo = 256 if rstr else 0
                            if rstr:
                                nc.gpsimd.memset(at[:, 2 * t:2 * t + 2, :lo], 0)
                            ps2 = ps2p.tile([P, 2, QB], F32, tag="ps2")
                            for u in (2 * t, 2 * t + 1):
                                nc.tensor.matmul(
                                    ps2[:, u - 2 * t, lo:],
                                    lhsT=kt[:, c0 + 8 * u:c0 + 8 * u + 8, :],
                                    rhs=qt[:, c0 + a0 + lo // 16:
                                           c0 + a0 + 32, :],
                                    start=True, stop=True,
                                )
                            # future vk transposes ride the score stream:
                            # qb0 preps octets 4..7, qb1 octets 8..11,
                            # qb2 octets 12..15, qb3 the next head's 0..3
                            if t == 1:
                                if qb < 3:
                                    emit_tr4(vk, hl, 4 * (qb + 1))
                                elif vk_next is not None:
                                    emit_tr4(vk_next, hl + 1, 0)
                            if t == 1:
                                # previous q-block's finalize rides here so the
                                # PE has fresh independent work queued first
                                flush_pending()
                            if t >= 2:
                                emit_av(2 * t - 4)
                                emit_av(2 * t - 3)
                            nc.scalar.activation(
                                at[:, 2 * t:2 * t + 2, lo:],
                                ps2[:, :, lo:], AF.Exp,
                                scale=1.0 / DH,
                            )
                            for u in (2 * t, 2 * t + 1):
                                if u >= 4 * qb:  # diagonal octet: causal mask
                                    nc.gpsimd.tensor_mul(
                                        at[:, u, lo:], at[:, u, lo:],
                                        mask_t[:, u - 4 * qb, lo:],
                                    )
                            if t == 0:
                                nc.vector.tensor_add(
                                    acc[:], at[:, 0, :], at[:, 1, :]
                                )
                            else:
                                nc.vector.tensor_add(
                                    acc[:, lo:], acc[:, lo:], at[:, 2 * t, lo:]
                                )
                                nc.vector.tensor_add(
                                    acc[:, lo:], acc[:, lo:],
                                    at[:, 2 * t + 1, lo:]
                                )
                        gqb += 1
                        emit_av(nk - 4)
                        emit_av(nk - 3)

                        def finalize(hl=hl, qb=qb, nk=nk, acc=acc,
                                     psum_o=psum_o, emit_av=emit_av):
                            emit_av(nk - 2)
                            emit_av(nk - 1)
                            psum_l = ps_lp.tile([P, QB], F32, tag="pl")
                            nc.tensor.matmul(
                                psum_l[:], lhsT=ones_t[:], rhs=acc[:],
                                start=True, stop=True,
                            )
                            # 1/l = exp(-ln(l)) on the scalar engine: both
                            # funcs live in one ACT table (no reload thrash),
                            # and this moves ~54us of DVE InstReciprocal
                            # (6.5ns/elem iterative) off the phase-B
                            # bottleneck engine.  |l| in [1, ~300] is safely
                            # inside the table domain; ~1e-3 rel err adds
                            # ~0.1% output error (gate is 2e-2).
                            nll = rcp.tile([P, QB], F16, tag="nll")
                            nc.scalar.activation(nll[:], psum_l[:], AF.Ln)
                            rcb = rcp.tile([P, QB], F32, tag="rcb")
                            nc.scalar.activation(rcb[:], nll[:], AF.Exp,
                                                 scale=-1.0)
                            nc.vector.tensor_mul(
                                att[hl][:, qb, :], psum_o[:], rcb[:]
                            )

                        pending.append(finalize)

                    vk = vk_next

                # ---- phase C: partial yT = WoT.T @ att, reusing B psum ----
                for sb in range(4):
                    for ot in range(N_DM):
                        pool = ps_op if (ot + sb) % 2 == 0 else ps_lp
                        psc = pool.tile([P, QB], F32,
                                        tag="po" if pool is ps_op else "pl")
                        for hl2 in range(HPC):
                            nc.tensor.matmul(
                                psc[:],
                                lhsT=wot_t[:, hl2, ot * P:(ot + 1) * P],
                                rhs=att[hl2][:, sb, :],
                                start=(hl2 == 0), stop=(hl2 == HPC - 1),
                            )
                        if sb == 0 and ot == 0:
                            flush_pending()
                        st = stg.tile([P, QB], F16, tag="st")
                        if (ot + sb) % 2 == 0:
                            nc.vector.tensor_copy(st[:], psc[:])
                        else:
                            nc.scalar.copy(st[:], psc[:])
                        nc.sync.dma_start(
                            yt_t3[:, ot, sb * QB:(sb + 1) * QB], st[:]
                        )

    _split_multi_waits(nc)
    return nc


_NC = None


def _make_masks():
    # causal masks for diagonal octets in (a-outer, r-inner) index order:
    # k partition i = 16a'' + r';  q column j = 16a_rel + r  (== position
    # within the q-block, so yt columns come out in plain s' order)
    # allow: 16*(8*delta + a'') + r' <= 16*a_rel + r
    k_lin = (16 * np.arange(8)[:, None] + np.arange(NR)[None, :]).reshape(-1)
    q_lin = (16 * np.arange(32)[:, None] + np.arange(NR)[None, :]).reshape(-1)
    out = np.empty((4, P, QB), dtype=np.float32)
    for d in range(4):
        out[d] = ((k_lin[:, None] + P * d) <= q_lin[None, :]).astype(np.float32)
    return out


def kernel(x, Wq, Wk, Wv, Wo, _want_trace=False, **_trace_kw):
    global _NC
    if _NC is None:
        _NC = _build_nc()
    nc = _NC

    import ml_dtypes
    f8 = ml_dtypes.float8_e4m3
    f16 = np.float16

    x = np.asarray(x, dtype=np.float32)
    Wq = np.asarray(Wq, dtype=np.float32)
    Wk = np.asarray(Wk, dtype=np.float32)
    Wv = np.asarray(Wv, dtype=np.float32)
    Wo = np.asarray(Wo, dtype=np.float32)

    wq8 = np.ascontiguousarray(Wq.T * SW).astype(f8)
    wk8 = np.ascontiguousarray(Wk.T * SW).astype(f8)
    wv16 = np.ascontiguousarray(Wv.T).astype(f16)
    masks = _make_masks().astype(f16)
    ones = np.ones((P, P), dtype=f16)
    ident = np.eye(P, dtype=np.float32).astype(f16)

    in_maps = []
    for c in range(N_CORES):
        b, g = divmod(c, HPC)
        sl = slice(g * DL, (g + 1) * DL)
        xs = np.ascontiguousarray(x[b, sl, :].T)
        in_maps.append({
            "x8": np.ascontiguousarray(xs * SX).astype(f8),
            "x16": xs.astype(f16),
            "wq8": wq8,
            "wk8": wk8,
            "wv16": wv16,
            "wot16": np.ascontiguousarray(Wo[:, sl].T).astype(f16),
            "maskc": masks,
            "ones": ones,
            "ident": ident,
        })

    res = run_bass_kernel_spmd(
        nc, in_maps, list(range(N_CORES)),
        trace=_want_trace, **_trace_kw,
    )

    y = np.empty((B, S, DM), dtype=np.float32)
    for b in range(B):
        acc = res.results[HPC * b]["yt"].astype(np.float32)
        for g in range(1, HPC):
            acc += res.results[HPC * b + g]["yt"].astype(np.float32)
        y[b] = acc.T
    if _want_trace:
        return y, res
    return y


# revision 25
# speedup vs baseline: 1.2501x; 1.0993x over previous
"""nn_MultiHeadAttention (B=2, S=2048, D=2048, H=16) on 8 NeuronCores.

The reference module splits heads with a plain reshape (no transpose):
    Q = (x @ Wq.T).reshape(B, H, S, Dh)
so head h attends over ROWS [128h, 128h+128) of Qmat = x @ Wq.T, with
attention position s' = 16a + r mapping to (row 128h + a, feature slice
[128r, 128r+128)).  The merge DOES transpose (standard), so
    y = sum_h outh @ Wo[:, 128h:128h+128].T.

Sharding: core c handles batch b=c//4 and head-group g=c%4 (heads
4g..4g+3, i.e. tokens [512g, 512g+512) of its batch).  Each core
computes those projection row-slices against the FULL Wq/Wk/Wv, causal
attention in the scrambled index space, and a partial output projection
against its column slice of Wo.  The host sums the 4 partials per batch
in fp32 and unscrambles the column order.

Precision strategy (validated numerically, rel-err ~2.5e-3 vs fp32):
  * Q/K projections run in fp8e4m3 with DoubleRow perf mode (two
    128-partition contraction subtiles per matmul = 2x PE throughput).
    Scales: x*32, W*1024 (both < 240 max-normal), descaled 2^-15 at the
    PSUM->SBUF copy.  Softmax forgives the ~2.6% Q/K quantization.
  * Everything else runs fp16 (NOT bf16): same PE speed, 4x lower
    rounding error, and 2x/4x DVE throughput for the elementwise work.
  * V path / attention weights / output projection must NOT be fp8
    (measured 2.4e-2..3.8e-2 rel-err = over the 2e-2 gate).

Layout: projections stored as single tiles [dh=128, r=16, 512 tokens]
filled by ONE copy per 512-wide PSUM stripe (no per-head scatter).
Head hl's tiles are column slices [:, :, 128*hl : 128*hl+128].  Scores
use k-octets (free index i = 8r' + a'') against 512-wide q-blocks
(j = 32r + a_rel); causal masks precomputed on host for this order.
Softmax denominators: DVE accumulates the exp'd octets per q-block
(fp16, 4x mode), then a single ones-matmul per q-block broadcasts the
partition sum - removing ~30us of ones-matmuls from the PE stream.
Per-q-block finalize (last attn@V pair, ones-matmul, reciprocal,
normalize) is deferred until the next q-block's first score pair so the
PE never stalls on the DVE chain.  exp runs on 2-octet batches to halve
the activation-engine instruction overhead (ACT is the phase-B
co-bottleneck).  The output projection reuses phase-B PSUM tiles and
streams each [128,512] block to DRAM as it completes.
"""

import sys

try:
    import concourse.bass as bass
except ImportError:  # harness may not have the repo on PYTHONPATH
    for p in ("/root/.axon_site", "/root/.axon_site/_ro/trn_rl_repo",
              "/root/.axon_site/_ro/pypackages", "/opt/trn_rl_repo"):
        if p not in sys.path:
            sys.path.append(p)
    import concourse.bass as bass

import numpy as np

import concourse.mybir as mybir
import concourse.tile as tile
from concourse.bass_utils import run_bass_kernel_spmd

F32 = mybir.dt.float32
F16 = mybir.dt.float16
F8 = mybir.dt.float8e4
AF = mybir.ActivationFunctionType
DR = mybir.MatmulPerfMode.DoubleRow

B = 2
S = 2048
DM = 2048
H = 16
DH = 128
N_CORES = 8
HPC = 4                 # heads per core
DL = HPC * DH           # 512: per-core token-slice width
P = 128
QB = 512                # q-block width = 32 a x 16 r
N_DM = DM // P          # 16 contraction subtiles
NR = 16                 # r-stripes per head

SX = 32.0               # fp8 scale on x       (|x|max ~5.3  -> ~170 < 240)
SW = 1024.0             # fp8 scale on Wq/Wk   (|W|max ~0.12 -> ~120 < 240)
DESCALE = 1.0 / (SX * SW)


def _split_multi_waits(nc):
    """This container's walrus rejects >1 sync-wait per instruction.
    Hoist extra waits onto same-engine NoOps inserted just before."""
    ctr = 0
    for f in nc.m.functions:
        for bb in f.blocks:
            insts = bb.instructions
            fixes = []
            for idx, inst in enumerate(insts):
                si = inst.sync_info
                ow = list(si.on_wait) if si and si.on_wait else []
                if len(ow) > 1:
                    fixes.append((idx, inst, ow, si))
            for idx, inst, ow, si in reversed(fixes):
                inst.sync_info = mybir.SyncInfo(on_wait=ow[-1:], on_update=si.on_update)
                for w in reversed(ow[:-1]):
                    ctr += 1
                    nop = mybir.InstNoOp(
                        name=f"I-waitsplit-{ctr}", engine=inst.engine, ins=[], outs=[]
                    )
                    nop.sync_info = mybir.SyncInfo(on_wait=[w], on_update=[])
                    nc.register_instruction(nop, overwrite=True)
                    insts.insert(idx, nop)
    return ctr


def _build_nc():
    nc = bass.Bass(target_bir_lowering=False)

    x8_d = nc.dram_tensor("x8", [DM, DL], F8, kind="ExternalInput")      # x[b,sl].T * 32
    x16_d = nc.dram_tensor("x16", [DM, DL], F16, kind="ExternalInput")   # x[b,sl].T
    wq8_d = nc.dram_tensor("wq8", [DM, DM], F8, kind="ExternalInput")    # Wq.T * 1024
    wk8_d = nc.dram_tensor("wk8", [DM, DM], F8, kind="ExternalInput")
    wv_d = nc.dram_tensor("wv16", [DM, DM], F16, kind="ExternalInput")   # Wv.T
    wot_d = nc.dram_tensor("wot16", [DL, DM], F16, kind="ExternalInput")  # Wo[:,sl].T
    mask_d = nc.dram_tensor("maskc", [4, P, QB], F16, kind="ExternalInput")
    ones_d = nc.dram_tensor("ones", [P, P], F16, kind="ExternalInput")
    ident_d = nc.dram_tensor("ident", [P, P], F16, kind="ExternalInput")
    yt_d = nc.dram_tensor("yt", [DM, S], F16, kind="ExternalOutput")     # partial y[b].T

    yt_t3 = yt_d.rearrange("(o p) s -> p o s", p=P)

    with tile.TileContext(nc) as tc:
        with (
            tc.tile_pool(name="proj", bufs=1) as proj,
            tc.tile_pool(name="bconst", bufs=1) as bconst,
            tc.tile_pool(name="attp", bufs=HPC) as attp,
            tc.tile_pool(name="wop", bufs=1) as wop,
            tc.tile_pool(name="stg", bufs=4) as stg,
            nc.allow_low_precision(reason="fp8/fp16 attention kernel"),
        ):
            # projection tiles [dh, r, token]; head hl = cols [128hl, 128hl+128)
            qt = proj.tile([P, DL, NR], F16, tag="qt")
            kt = proj.tile([P, DL, NR], F16, tag="kt")
            vt = proj.tile([P, DL, NR], F16, tag="vt")
            # normalized attention outputs per head [dh, qb, j]
            att = [attp.tile([P, 4, QB], F16, tag="att", name=f"att{i}")
                   for i in range(HPC)]
            ones_t = bconst.tile([P, P], F16, tag="ones")
            ident_t = bconst.tile([P, P], F16, tag="ident")
            mask_t = bconst.tile([P, 4, QB], F16, tag="mask")
            wot_t = wop.tile([P, HPC, DM], F16, tag="wo")

            # ---- phase A: projections straight into SBUF ----
            with (
                tc.tile_pool(name="xp", bufs=1) as xp,
                tc.tile_pool(name="wp", bufs=6) as wp,
                tc.tile_pool(name="ps_a", bufs=3, space="PSUM") as ps_a,
            ):
                x8_t = xp.tile([P, N_DM, DL], F8, tag="x8")
                x16_t = xp.tile([P, N_DM, DL], F16, tag="x16")
                x8_t3 = x8_d.rearrange("(o p) s -> p o s", p=P)
                nc.sync.dma_start(x8_t[:, 0:8, :], x8_t3[:, 0:8, :])
                x16_t3 = x16_d.rearrange("(o p) s -> p o s", p=P)

                for w_i, (w_d, w_dt, dst) in enumerate((
                    (wq8_d, F8, qt),
                    (wk8_d, F8, kt),
                    (wv_d, F16, vt),
                )):
                    w_t3 = w_d.rearrange("(o p) d -> p o d", p=P)
                    # stripe PAIRS share a 2-bank psum tile so the strided
                    # scatter copy writes 4-byte token units (the 2-byte
                    # strided write pattern measured 3x slower on hw)
                    for rp in range(NR // 2):
                        if w_i == 1 and rp < 4:
                            # x16 streamed in chunks between K stripes so it
                            # doesn't stall the K weight stream
                            nc.sync.dma_start(
                                x16_t[:, 4 * rp:4 * rp + 4, :],
                                x16_t3[:, 4 * rp:4 * rp + 4, :],
                            )
                        psum = ps_a.tile([P, 2, QB], F32, tag="pa")
                        for half in range(2):
                            rt = 2 * rp + half
                            w_t = wp.tile([P, N_DM, P], w_dt, tag=f"w{w_i}")
                            nc.sync.dma_start(
                                w_t[:], w_t3[:, :, rt * P:(rt + 1) * P]
                            )
                            if w_i == 0 and rp == 0 and half == 0:
                                # x8 upper half queued right behind the first
                                # Q stripe so the first matmuls start earlier
                                nc.sync.dma_start(
                                    x8_t[:, 8:16, :], x8_t3[:, 8:16, :]
                                )
                            if w_dt == F8:
                                for d in range(8):
                                    nc.tensor.matmul(
                                        psum[:, half, :],
                                        lhsT=w_t[:, 2 * d:2 * d + 2, :],
                                        rhs=x8_t[:, 2 * d:2 * d + 2, :],
                                        start=(d == 0), stop=(d == 7),
                                        perf_mode=DR,
                                    )
                            else:
                                for d in range(N_DM):
                                    nc.tensor.matmul(
                                        psum[:, half, :],
                                        lhsT=w_t[:, d, :], rhs=x16_t[:, d, :],
                                        start=(d == 0), stop=(d == N_DM - 1),
                                    )
                        # one paired scatter copy, spread across engines
                        dst_ap = dst[:, :, 2 * rp:2 * rp + 2]
                        src_ap = psum[:].rearrange("p t c -> p c t")
                        if w_i == 0 or (w_i == 2 and rp % 2 == 0):
                            nc.vector.tensor_scalar_mul(
                                dst_ap, src_ap, DESCALE if w_i == 0 else 1.0
                            )
                        elif w_i == 1:
                            nc.scalar.mul(dst_ap, src_ap, DESCALE)
                        else:
                            nc.scalar.copy(dst_ap, src_ap)

            # constants + Wo stream in behind the weight DMAs
            nc.sync.dma_start(ident_t[:], ident_d[:])
            nc.sync.dma_start(ones_t[:], ones_d[:])
            nc.sync.dma_start(mask_t[:], mask_d.rearrange("c p q -> p c q"))
            nc.sync.dma_start(wot_t[:], wot_d.rearrange("(hl p) d -> p hl d", p=P))

            # ---- phase B + C ----
            with (
                tc.tile_pool(name="atp", bufs=2) as atp,
                tc.tile_pool(name="accp", bufs=2) as accp,
                tc.tile_pool(name="vkp", bufs=2) as vkp,
                tc.tile_pool(name="rcp", bufs=2) as rcp,
                tc.tile_pool(name="ps2", bufs=2, space="PSUM") as ps2p,
                tc.tile_pool(name="ps_o", bufs=2, space="PSUM") as ps_op,
                tc.tile_pool(name="ps_l", bufs=1, space="PSUM") as ps_lp,
                tc.tile_pool(name="ps_t", bufs=1, space="PSUM") as ps_tp,
            ):
                pend_a = []   # deferred last attn@V pairs
                pend_b = []   # deferred denominator chains
                gqb = 0

                def flush_pending(q):
                    while q:
                        q.pop(0)()

                def emit_tr4(vk_dst, src_hl, m0):
                    # k-major V tiles via PE transpose: vk[i=8r'+a'', m, dh].
                    # 4 transposes share one PSUM bank; one DVE copy drains it
                    # (GPSIMD cannot read PSUM on this target).
                    ps_t = ps_tp.tile([P, 4, P], F16, tag="pt")
                    for k in range(4):
                        cc = src_hl * P + 8 * (m0 + k)
                        nc.tensor.transpose(
                            ps_t[:, k, :], vt[:, cc:cc + 8, :], ident_t[:]
                        )
                    nc.vector.tensor_copy(vk_dst[:, m0:m0 + 4, :], ps_t[:])

                vk = vkp.tile([P, NR, P], F16, tag="vk", name="vk0")
                emit_tr4(vk, 0, 0)

                for hl in range(HPC):
                    c0 = hl * P
                    vk_next = (vkp.tile([P, NR, P], F16, tag="vk",
                                        name=f"vk{hl + 1}")
                               if hl + 1 < HPC else None)

                    for qb in range(4):
                        nk = 4 * qb + 4
                        npair = nk // 2
                        a0 = 32 * qb
                        at = atp.tile([P, NR, QB], F16, tag="at")
                        acc_a = accp.tile([P, QB], F16, tag="acc", name="acca")
                        acc_b = accp.tile([P, QB], F16, tag="acc", name="accb")
                        psum_o = ps_op.tile([P, QB], F32, tag="po")

                        def emit_av(u, vk=vk, at=at, psum_o=psum_o, nk=nk):
                            nc.tensor.matmul(
                                psum_o[:], lhsT=vk[:, u, :], rhs=at[:, u, :],
                                start=(u == 0), stop=(u == nk - 1),
                            )

                        for t in range(npair):
                            # the last pair holds diagonal octets 4qb+2/4qb+3
                            # whose valid q-columns are [256, 512): compute
                            # only those, zero the dead half explicitly
                            rstr = t == npair - 1
                            lo = 256 if rstr else 0
                            if rstr:
                                nc.gpsimd.memset(at[:, 2 * t:2 * t + 2, :lo], 0)
                            ps2 = ps2p.tile([P, 2, QB], F32, tag="ps2")
                            for u in (2 * t, 2 * t + 1):
                                nc.tensor.matmul(
                                    ps2[:, u - 2 * t, lo:],
                                    lhsT=kt[:, c0 + 8 * u:c0 + 8 * u + 8, :],
                                    rhs=qt[:, c0 + a0 + lo // 16:
                                           c0 + a0 + 32, :],
                                    start=True, stop=True,
                                )
                            # future vk transposes ride the score stream:
                            # qb0 preps octets 4..7, qb1 octets 8..11,
                            # qb2 octets 12..15, qb3 the next head's 0..3
                            if t == 1:
                                if qb < 3:
                                    emit_tr4(vk, hl, 4 * (qb + 1))
                                elif vk_next is not None:
                                    emit_tr4(vk_next, hl + 1, 0)
                            # previous q-block's finalize is split: its last
                            # attn@V pair flushes early (gated on fast
                            # ACT/Pool work), the denominator chain flushes on
                            # the LAST pair so the DVE add-chain has a whole
                            # q-block of slack before the PE waits on it
                            if t == 1:
                                flush_pending(pend_a)
                            if t == npair - 1:
                                flush_pending(pend_b)
                            if t >= 2:
                                emit_av(2 * t - 4)
                                emit_av(2 * t - 3)
                            nc.scalar.activation(
                                at[:, 2 * t:2 * t + 2, lo:],
                                ps2[:, :, lo:], AF.Exp,
                                scale=1.0 / DH,
                            )
                            for u in (2 * t, 2 * t + 1):
                                if u >= 4 * qb:
                                    # causal mask: only columns [128d, 128d+128)
                                    # are partial; below them at must be zero,
                                    # above them the mask is all-ones
                                    dd = u - 4 * qb
                                    ms = 128 * dd
                                    if dd == 1:
                                        nc.gpsimd.memset(at[:, u, 0:128], 0)
                                    elif dd == 3:
                                        nc.gpsimd.memset(at[:, u, 256:384], 0)
                                    nc.gpsimd.tensor_mul(
                                        at[:, u, ms:ms + 128],
                                        at[:, u, ms:ms + 128],
                                        mask_t[:, dd, ms:ms + 128],
                                    )
                            # ping-pong accumulator (in-place add measured 1x
                            # on DVE; alternating destinations allows 2x)
                            if t == 0:
                                nc.vector.tensor_add(
                                    acc_a[:], at[:, 0, :], at[:, 1, :]
                                )
                            else:
                                nc.vector.tensor_add(
                                    acc_b[:], acc_a[:], at[:, 2 * t, :]
                                )
                                nc.vector.tensor_add(
                                    acc_a[:], acc_b[:], at[:, 2 * t + 1, :]
                                )
                        gqb += 1
                        emit_av(nk - 4)
                        emit_av(nk - 3)

                        def fin_avs(nk=nk, emit_av=emit_av):
                            emit_av(nk - 2)
                            emit_av(nk - 1)

                        def fin_den(hl=hl, qb=qb, acc=acc_a, psum_o=psum_o):
                            psum_l = ps_lp.tile([P, QB], F32, tag="pl")
                            nc.tensor.matmul(
                                psum_l[:], lhsT=ones_t[:], rhs=acc[:],
                                start=True, stop=True,
                            )
                            # 1/l = exp(-ln(l)) on the scalar engine: both
                            # funcs live in one ACT table (no reload thrash);
                            # DVE InstReciprocal measured 3.35us per call.
                            # |l| in [1, ~300] is in-domain; ~1e-3 rel err
                            # adds ~0.1% output error (gate is 2e-2).
                            nll = rcp.tile([P, QB], F16, tag="nll")
                            nc.scalar.activation(nll[:], psum_l[:], AF.Ln)
                            rcb = rcp.tile([P, QB], F32, tag="rcb")
                            nc.scalar.activation(rcb[:], nll[:], AF.Exp,
                                                 scale=-1.0)
                            nc.vector.tensor_mul(
                                att[hl][:, qb, :], psum_o[:], rcb[:]
                            )

                        pend_a.append(fin_avs)
                        pend_b.append(fin_den)

                    vk = vk_next

                # ---- phase C: partial yT = WoT.T @ att, reusing B psum ----
                for sb in range(4):
                    for ot in range(N_DM):
                        pool = ps_op if (ot + sb) % 2 == 0 else ps_lp
                        psc = pool.tile([P, QB], F32,
                                        tag="po" if pool is ps_op else "pl")
                        for hl2 in range(HPC):
                            nc.tensor.matmul(
                                psc[:],
                                lhsT=wot_t[:, hl2, ot * P:(ot + 1) * P],
                                rhs=att[hl2][:, sb, :],
                                start=(hl2 == 0), stop=(hl2 == HPC - 1),
                            )
                        if sb == 0 and ot == 0:
                            flush_pending(pend_a)
                            flush_pending(pend_b)
                        st = stg.tile([P, QB], F16, tag="st")
                        if (ot + sb) % 2 == 0:
                            nc.vector.tensor_copy(st[:], psc[:])
                        else:
                            nc.scalar.copy(st[:], psc[:])
                        nc.sync.dma_start(
                            yt_t3[:, ot, sb * QB:(sb + 1) * QB], st[:]
                        )

    _split_multi_waits(nc)
    return nc


_NC = None


def _make_masks():
    # causal masks for diagonal octets in (a-outer, r-inner) index order:
    # k partition i = 16a'' + r';  q column j = 16a_rel + r  (== position
    # within the q-block, so yt columns come out in plain s' order)
    # allow: 16*(8*delta + a'') + r' <= 16*a_rel + r
    k_lin = (16 * np.arange(8)[:, None] + np.arange(NR)[None, :]).reshape(-1)
    q_lin = (16 * np.arange(32)[:, None] + np.arange(NR)[None, :]).reshape(-1)
    out = np.empty((4, P, QB), dtype=np.float32)
    for d in range(4):
        out[d] = ((k_lin[:, None] + P * d) <= q_lin[None, :]).astype(np.float32)
    return out


def kernel(x, Wq, Wk, Wv, Wo, _want_trace=False, **_trace_kw):
    global _NC
    if _NC is None:
        _NC = _build_nc()
    nc = _NC

    import ml_dtypes
    f8 = ml_dtypes.float8_e4m3
    f16 = np.float16

    x = np.asarray(x, dtype=np.float32)
    Wq = np.asarray(Wq, dtype=np.float32)
    Wk = np.asarray(Wk, dtype=np.float32)
    Wv = np.asarray(Wv, dtype=np.float32)
    Wo = np.asarray(Wo, dtype=np.float32)

    wq8 = np.ascontiguousarray(Wq.T * SW).astype(f8)
    wk8 = np.ascontiguousarray(Wk.T * SW).astype(f8)
    wv16 = np.ascontiguousarray(Wv.T).astype(f16)
    masks = _make_masks().astype(f16)
    ones = np.ones((P, P), dtype=f16)
    ident = np.eye(P, dtype=np.float32).astype(f16)

    in_maps = []
    for c in range(N_CORES):
        b, g = divmod(c, HPC)
        sl = slice(g * DL, (g + 1) * DL)
        xs = np.ascontiguousarray(x[b, sl, :].T)
        in_maps.append({
            "x8": np.ascontiguousarray(xs * SX).astype(f8),
            "x16": xs.astype(f16),
            "wq8": wq8,
            "wk8": wk8,
            "wv16": wv16,
            "wot16": np.ascontiguousarray(Wo[:, sl].T).astype(f16),
            "maskc": masks,
            "ones": ones,
            "ident": ident,
        })

    res = run_bass_kernel_spmd(
        nc, in_maps, list(range(N_CORES)),
        trace=_want_trace, **_trace_kw,
    )

    y = np.empty((B, S, DM), dtype=np.float32)
    for b in range(B):
        acc = res.results[HPC * b]["yt"].astype(np.float32)
        for g in range(1, HPC):
            acc += res.results[HPC * b + g]["yt"].astype(np.float32)
        y[b] = acc.T
    if _want_trace:
        return y, res
    return y
